# revision 66
# baseline (speedup 1.0000x reference)
"""Multi-head attention (B=4, S=2048, D=1024, H=16, HD=64) on 8 TRN2 NeuronCores.

Sharding: core c handles batch b = c//2 and head-group hg = c%2 (8 heads each).
Attention is embarrassingly parallel over (b, head-group); the QKV projection is
column-sharded per core (tensor parallel on heads).

Per-core dataflow:
  - Host passes X^T [D, S] (f16), W slices in permuted [D, cols] f16 layout.
  - Projection:  Q^T/K^T [1024, S] = W_qk^T @ X accumulated into SBUF f32r
                 tiles (sbt) read directly by the attention stage;
                 V' [S, 8, 64+1] = X @ W_v in f16 with a ones column per head.
  - Per head, per 128-row chunk of S^T: 4 QK matmuls write four single-bank
                 [128,512] PSUM tiles from a 5-deep pool, so the next chunk's
                 QK never waits on this chunk's exp (the recycle distance is
                 >1 chunk). st = exp(S^T/8) in f16: quarters 0/2 on ScalarE,
                 quarters 1/3 via DVE copies into one staging tile and a
                 single contiguous Pool-engine pow(e^(1/8), x) — st stores its
                 quarters in block order (q0,q2,q1,q3) so the pow writes one
                 contiguous range (strided writes mis-track subtile deps).
                 The three producers run concurrently so no engine paces the
                 loop; each head's last chunk runs entirely on ScalarE so the
                 pool/DVE chain never tails past the head boundary.
                 out[q, d] += st.T @ V with st as the matmul STATIONARY
                 operand and V [128,64] f16 moving (64-row matmuls instead of
                 512). Softmax sums accumulate via ap-1 matmuls against the
                 V' ones column (same stationary).
                 PSUM discipline: an accumulator bank is zeroed by the single
                 start=True matmul that first touches it; everything else
                 accumulates start=False (a start mid-stream would zero the
                 whole bank). AV numerators: one 2-bank tile; sums: one bank.
  - Normalization happens on the HOST: the kernel outputs the numerators
    [S, 512] and the softmax sums [S, 8]; assemble() divides.

The Q/K projection is sliced into full-depth quanta woven into the attention
chunk loop: Q-side m-tiles complete before their head pair starts; K-side
m-tiles are produced just-in-time INSIDE the first consuming head (chunk kc of
K is only read at slot kc), spreading PE filler work across heads 1-6. The V
projection and m-tiles 0/4 run as a prefix behind the X^T DMA stream. AV
quarter-chunks are emitted from a ready-queue one slot behind ScalarE pieces
and three slots behind Pool pieces; per-head readout is split into DVE pieces
woven into the next head's first slots.
"""

import numpy as np

import concourse.bass as bass
import concourse.mybir as mybir
import concourse.tile as tile
from concourse import bacc
from concourse.bass import AP
from concourse.bass_utils import run_bass_kernel_spmd

F32 = mybir.dt.float32
F32R = mybir.dt.float32r
F16 = mybir.dt.float16
AF = mybir.ActivationFunctionType
ALU = mybir.AluOpType

P = 128          # partitions
D = 1024         # model dim
S = 2048         # sequence
HD = 64          # head dim
NHC = 8          # heads per core
QKC = NHC * HD   # 512 columns per core for each of Q, K, V
KD = D // P      # 8 contraction chunks
MS = S // P      # 16 sequence chunks
QC = S // P      # 16 q-chunks of 128
SCALE = 1.0 / 8.0  # 1/sqrt(HD)
EBASE = float(np.exp(SCALE))

N_CORES = 8
B_FULL, H_FULL = 4, 16

SC_LAG = 1       # slots before a ScalarE-produced st piece may feed AV
POOL_LAG = 4     # slots before a Pool-produced st piece may feed AV


def _bcast(ap, n):
    """Append a stride-0 dim of size n to an AP (free-dim broadcast)."""
    return AP(ap.tensor, ap.offset, [*ap.ap, [0, n]])


def _build(iters=1):
    nc = bacc.Bacc(None, target_bir_lowering=False)

    xt = nc.dram_tensor("xt", [D, S], F16, kind="ExternalInput")
    # wqk is host-permuted: row (m*128 + p), col (k*128 + j) holds
    # W_qk[k*128 + p, m*128 + j] — one m-tile's weights contiguous per row
    wqk = nc.dram_tensor("wqk", [D, 2 * QKC], F16, kind="ExternalInput")
    wv = nc.dram_tensor("wv", [D, QKC], F16, kind="ExternalInput")
    bqk = nc.dram_tensor("bqk", [2 * QKC], F32, kind="ExternalInput")
    bv = nc.dram_tensor("bv", [QKC], F32, kind="ExternalInput")
    outd = nc.dram_tensor("out", [S, QKC], F16, kind="ExternalOutput")
    sumd = nc.dram_tensor("sums", [S, NHC], F32, kind="ExternalOutput")

    with tile.TileContext(nc) as tc:
        with (
            tc.tile_pool(name="persist", bufs=1) as pp,
            tc.tile_pool(name="sbtp", bufs=4) as sbtp,
            tc.tile_pool(name="stp", bufs=8) as stp,
            tc.tile_pool(name="stgp", bufs=6) as stgp,
            tc.tile_pool(name="outp", bufs=2) as outp,
            tc.tile_pool(name="smop", bufs=2) as smop,
            tc.tile_pool(name="psc", bufs=5, space="PSUM") as psc,
            tc.tile_pool(name="psav", bufs=1, space="PSUM") as psav,
            tc.tile_pool(name="psm", bufs=1, space="PSUM") as psm,
        ):
            # bias staging: bqk_sb[p, m] = bqk[m*128 + p]; bv broadcast.
            # Issued from the Pool sequencer so they don't delay the xt/w
            # stream on SP (they are only needed by the first DVE adds).
            bqk_sb = pp.tile([P, KD], F32, tag="bqk", name="bqk_sb")
            nc.gpsimd.dma_start(out=bqk_sb[:], in_=bqk[:].rearrange("(m p) -> p m", p=P))
            bv_bc = pp.tile([P, QKC], F32, tag="bvb", name="bv_bc")
            nc.gpsimd.dma_start(out=bv_bc[0:1, :], in_=bv[:].rearrange("(o n) -> o n", o=1))
            nc.gpsimd.partition_broadcast(bv_bc[:], bv_bc[0:1, :])
            ones8 = pp.tile([P, NHC], F32, tag="ones8", name="ones8")
            nc.vector.memset(ones8[:], 1.0)
            ebase = pp.tile([P, 1], F32, tag="eb", name="ebase")
            nc.vector.memset(ebase[:], EBASE)
            eb2 = _bcast(_bcast(ebase[:, 0:1], 2), 512)

            for it in range(iters):
                # V' tiles: [128 seq, 8 heads, 64+1] f16, ones in last column
                v_sb = [
                    pp.tile([P, NHC, HD + 1], F16, tag=f"v{k}", name=f"v{it}_{k}")
                    for k in range(MS)
                ]

                with tc.tile_pool(name=f"proj{it}", bufs=1) as pj:
                    w_tiles = {}

                    def load_wm(m, it=it):
                        w_tiles[m] = pj.tile([P, KD, P], F16, tag="wm", bufs=4,
                                             name=f"wm{it}_{m}")
                        nc.sync.dma_start(
                            out=w_tiles[m][:],
                            in_=wqk[m * P:(m + 1) * P, :].rearrange("p (k j) -> p k j", k=KD))

                    xt_sb = [pj.tile([P, S], F16, tag=f"xt{k}", name=f"xt{it}_{k}")
                             for k in range(KD)]
                    wv_sb = [pj.tile([P, QKC], F16, tag=f"wv{k}", name=f"wv{it}_{k}")
                             for k in range(KD)]
                    # issue order: first QK weight halves interleaved with xt
                    # chunk-0 quarters, so the k-major prefix starts earliest
                    w_tiles[0] = pj.tile([P, KD, P], F16, tag="wm", bufs=4,
                                         name=f"wm{it}_0")
                    wq0 = wqk[0:P, :].rearrange("p (k j) -> p k j", k=KD)
                    nc.sync.dma_start(out=w_tiles[0][:, 0:2, :], in_=wq0[:, 0:2, :])
                    nc.sync.dma_start(out=xt_sb[0][:, 0:1024], in_=xt[0:P, 0:1024])
                    nc.sync.dma_start(out=w_tiles[0][:, 2:KD, :], in_=wq0[:, 2:KD, :])
                    nc.sync.dma_start(out=xt_sb[0][:, 1024:2048], in_=xt[0:P, 1024:2048])
                    load_wm(4)
                    for k in range(1, KD):
                        nc.sync.dma_start(out=xt_sb[k][:], in_=xt[k * P:(k + 1) * P, :])
                    for k in range(KD):
                        nc.sync.dma_start(out=wv_sb[k][:], in_=wv[k * P:(k + 1) * P, :])

                    sbt_tiles = {}

                    def new_sbt(m, it=it):
                        sbt_tiles[m] = sbtp.tile([P, S], F32R, tag="sbt",
                                                 name=f"sbt{it}_{m}")

                    def qk_quantum_fd(m, quarter, it=it):
                        """Full-depth quantum: one 512-wide quarter of m-tile m,
                        all 8 contraction chunks in one PSUM group, one DVE add."""
                        if m not in sbt_tiles:
                            new_sbt(m)
                        w_m, sbt = w_tiles[m], sbt_tiles[m]
                        ps = psc.tile([P, 512], F32, tag="sc", name=f"pq{it}_{m}_{quarter}")
                        for k in range(KD):
                            nc.tensor.matmul(
                                ps[:], w_m[:, k, :],
                                xt_sb[k][:, quarter * 512:(quarter + 1) * 512],
                                start=(k == 0), stop=(k == KD - 1))
                        nc.vector.tensor_scalar_add(
                            sbt[:, quarter * 512:(quarter + 1) * 512], ps[:],
                            bqk_sb[:, m:m + 1])

                    def prefix_m0_m4(it=it):
                        """m-tiles 0 and 4 together, k-major across 8 concurrent
                        single-bank PSUM groups (4 sc + av-as-2 + sm + sc) —
                        DMA-paced trickle behind the xt stream. The last two
                        contraction chunks run group-by-group so the group
                        completions (and their DVE adds) stagger instead of
                        bunching into one serial DVE burst at the end."""
                        for m in (0, 4):
                            new_sbt(m)
                        g_m0 = [psc.tile([P, 512], F32, tag="sc", name=f"pa{it}_{q}")
                                for q in range(4)]
                        g_m4a = psav.tile([P, 1024], F32, tag="av", name=f"pb{it}")
                        g_m4b = psm.tile([P, 512], F32, tag="sm", name=f"pc{it}")
                        g_m4c = psc.tile([P, 512], F32, tag="sc", name=f"pd{it}")
                        # (dest-512-slice, weight m-tile, xt column block, sbt slice, bias col)
                        # head 0's first QK reads kt chunk 0 (m4 cols 0:512)
                        # and all of qt (m0): finish those groups first
                        groups = (
                            [(g_m4a[:, 0:512], 4, 0)]
                            + [(g_m0[q][:], 0, q) for q in range(4)]
                            + [(g_m4a[:, 512:1024], 4, 1),
                               (g_m4b[:], 4, 2), (g_m4c[:], 4, 3)]
                        )
                        for k in range(KD - 2):
                            for dst, m, q in groups:
                                nc.tensor.matmul(
                                    dst, w_tiles[m][:, k, :],
                                    xt_sb[k][:, q * 512:(q + 1) * 512],
                                    start=(k == 0), stop=False)
                        for dst, m, q in groups:
                            for k in (KD - 2, KD - 1):
                                nc.tensor.matmul(
                                    dst, w_tiles[m][:, k, :],
                                    xt_sb[k][:, q * 512:(q + 1) * 512],
                                    start=False, stop=(k == KD - 1))
                            nc.vector.tensor_scalar_add(
                                sbt_tiles[m][:, q * 512:(q + 1) * 512], dst,
                                bqk_sb[:, m:m + 1])

                    def v_quantum(ms, it=it, v_sb=v_sb, steady=False):
                        """Full-depth V' projection for sequence chunk ms (all 8
                        heads, N=512). Steady quanta use the sc pool; the two
                        prefix quanta reuse the sm bank."""
                        pool = psc if steady else psm
                        ps = pool.tile([P, QKC], F32, tag="sc" if steady else "sm",
                                       name=f"pv{it}_{ms}")
                        for k in range(KD):
                            nc.tensor.matmul(
                                ps[:], xt_sb[k][:, ms * P:(ms + 1) * P], wv_sb[k][:],
                                start=(k == 0), stop=(k == KD - 1))
                        dst = v_sb[ms][:, :, 0:HD]
                        nc.vector.tensor_tensor(
                            out=dst, in0=ps[:].rearrange("p (h e) -> p h e", e=HD),
                            in1=bv_bc[:, :].rearrange("p (h e) -> p h e", e=HD),
                            op=ALU.add)
                        nc.vector.tensor_copy(
                            v_sb[ms][:, :, HD:HD + 1],
                            ones8[:, :].rearrange("p (h o) -> p h o", o=1))

                    # ---- static schedule of projection/V quanta ----
                    sched = {}

                    def add(h, kc, fn):
                        sched.setdefault((h, kc), []).append(fn)

                    # all V chunks just-in-time inside head 0 (chunk ms is
                    # first consumed by AV at slot >= 3)
                    for ms in range(MS):
                        add(0, max(ms - 2, 0) if ms >= 2 else ms,
                            lambda ms=ms: v_quantum(ms, steady=True))
                    # Q-side m-tiles (m1/m2/m3) complete in the head before
                    # their pair; K-side m-tiles (m5/m6/m7) stream just-in-time
                    # into the first consuming head (chunk kc read at slot kc).
                    for i, p in enumerate((1, 2, 3)):
                        hq, hk = 2 * p - 1, 2 * p
                        add(hq - 1, 8, lambda m=p: load_wm(m))
                        add(hq, 4, lambda m=4 + p: load_wm(m))
                        for q in range(4):
                            add(hq, 4 * q + 1, lambda m=p, q=q: qk_quantum_fd(m, q))
                        add(hq, 14, lambda m=4 + p: qk_quantum_fd(m, 0))
                        for q in range(1, 4):
                            add(hk, 4 * q - 2, lambda m=4 + p, q=q: qk_quantum_fd(m, q))

                    # prefix: m-tiles 0/4 trickling behind the xt load
                    prefix_m0_m4()

                    # ---------------- attention ----------------
                    def make_readout(h, avps, smps, it=it):
                        """PSUM -> SBUF stage -> DRAM readout for head h, split
                        into two DVE pieces woven into the next head's early
                        chunk slots. Host normalizes."""
                        out_sb = outp.tile([P, QC, HD], F16, tag="osb",
                                           name=f"osb{it}_{h}")
                        sm_sb = smop.tile([P, QC], F32, tag="smo", name=f"smo{it}_{h}")

                        def piece(b, quarter=False):
                            w = 4 if quarter else 8
                            if quarter and b % 2:
                                nc.scalar.activation(
                                    out_sb[:, w * b:w * b + w, :],
                                    avps[:, w * b:w * b + w, :], AF.Copy)
                            else:
                                nc.vector.tensor_copy(
                                    out_sb[:, w * b:w * b + w, :],
                                    avps[:, w * b:w * b + w, :])
                            last = b == (3 if quarter else 1)
                            if last:
                                nc.vector.tensor_copy(sm_sb[:], smps[:])
                            eng = (nc.scalar if b % 2 and h == NHC - 1 else nc.sync)
                            eng.dma_start(
                                out=outd[:, h * HD:(h + 1) * HD].rearrange(
                                    "(c p) d -> p c d", p=P)[:, w * b:w * b + w, :],
                                in_=out_sb[:, w * b:w * b + w, :])
                            if last:
                                nc.sync.dma_start(
                                    out=sumd[:, h:h + 1].rearrange(
                                        "(c p) o -> p (c o)", p=P),
                                    in_=sm_sb[:])

                        return (lambda: piece(0)), (lambda: piece(1)), piece

                    def attention_head(h, carry, it=it, v_sb=v_sb):
                        g = h // 2
                        off = (h % 2) * HD
                        qt = sbt_tiles[g]
                        kt = sbt_tiles[4 + g]

                        avps = psav.tile([P, QC, HD], F32, tag="av",
                                         name=f"av{it}_{h}")
                        smps = psm.tile([P, QC], F32, tag="sm", name=f"sm{it}_{h}")
                        emits = [0, 0, 0, 0]       # emissions done per quarter
                        bank_started = [False, False]
                        sums_started = [False]

                        def emit_av_piece(kc, piece, st):
                            emits[piece] += 1
                            last = emits[piece] == MS
                            bank = piece // 2
                            blk = (0, 2, 1, 3)[piece]
                            for qc in range(piece * 4, piece * 4 + 4):
                                stat = st[:, blk * 512 + (qc % 4) * P:
                                          blk * 512 + (qc % 4 + 1) * P]
                                nc.tensor.matmul(
                                    avps[:, qc, :], stat,
                                    v_sb[kc][:, h, 0:HD],
                                    start=(not bank_started[bank]),
                                    stop=last)
                                bank_started[bank] = True
                                nc.tensor.matmul(
                                    smps[:, qc:qc + 1], stat,
                                    v_sb[kc][:, h, HD:HD + 1],
                                    start=(not sums_started[0]),
                                    stop=last)
                                sums_started[0] = True

                        # first slot on which this head's own AV may start:
                        # slot 3 leaves room for the previous head's carried
                        # AV pieces (slots 0-1) and readout (slots 1-2)
                        first_av = 1 if h == 0 else 3

                        pending = []  # (ready_slot, kc, piece, st)
                        for kc in range(MS):
                            # previous head's carried AV pieces lead slots 0-1:
                            # bank A at slot 0, bank B at slot 1, so each bank
                            # fully accumulates before its readout piece runs
                            if kc < 2 and carry:
                                for fn in carry.pop(0):
                                    fn()
                            st = stp.tile([P, S], F16, tag="st",
                                          name=f"st{it}_{h}_{kc}")
                            scs = []
                            for q in range(4):
                                sc = psc.tile([P, 512], F32, tag="sc",
                                              name=f"sc{it}_{h}_{kc}_{q}")
                                nc.tensor.matmul(
                                    sc[:],
                                    kt[off:off + HD, kc * P:(kc + 1) * P],
                                    qt[off:off + HD, q * 512:(q + 1) * 512],
                                    start=True, stop=True)
                                scs.append(sc)
                            # each head's last chunk (h7: last two) runs fully
                            # on ScalarE so the pool/DVE chain never tails past
                            # the head boundary and carried pieces are ready
                            all_sc = kc == MS - 1 or (h == NHC - 1 and kc == MS - 2)
                            sc_qs = (0, 1, 2, 3) if all_sc else (0, 2)
                            for q in sc_qs:
                                blk = (0, 2, 1, 3)[q]
                                nc.scalar.activation(
                                    st[:, blk * 512:(blk + 1) * 512], scs[q][:],
                                    AF.Exp, scale=SCALE)
                                pending.append(
                                    (max(kc + SC_LAG, first_av), kc, q, st))
                            if not all_sc:
                                # quarters 1/3: DVE copies + one contiguous
                                # Pool pow into st blocks 2-3
                                stg = stgp.tile([P, 2, 512], F32, tag="stg",
                                                name=f"stg{it}_{h}_{kc}")
                                nc.vector.tensor_copy(stg[:, 0, :], scs[1][:])
                                nc.vector.tensor_copy(stg[:, 1, :], scs[3][:])
                                nc.gpsimd.tensor_tensor(
                                    out=st[:, 1024:2048].rearrange(
                                        "p (b x) -> p b x", x=512),
                                    in0=eb2, in1=stg[:], op=ALU.pow)
                                pending.append(
                                    (max(kc + POOL_LAG, first_av), kc, 1, st))
                                pending.append(
                                    (max(kc + POOL_LAG, first_av), kc, 3, st))
                            due = [x for x in pending if x[0] <= kc]
                            for x in due:
                                pending.remove(x)
                                emit_av_piece(x[1], x[2], x[3])
                            for fn in sched.pop((h, kc), ()):
                                fn()
                        piece0, piece1, piece = make_readout(h, avps, smps)
                        if h + 1 < NHC:
                            # within each bank, ready (ScalarE) pieces first: a
                            # carried pool piece still in flight would block
                            # the in-order PE stream at the next head's start
                            pending.sort(key=lambda x: (x[2] // 2, x[0], x[2]))

                            def mk(x):
                                return (lambda kc=x[1], piece=x[2], st=x[3]:
                                        emit_av_piece(kc, piece, st))

                            # slot 0: bank-A pieces already produced; slot 1:
                            # bank-A stragglers (still in flight) then bank B —
                            # both banks complete before their readout piece
                            carry_out = [
                                [mk(x) for x in pending
                                 if x[2] // 2 == 0 and x[0] <= MS],
                                [mk(x) for x in pending
                                 if x[2] // 2 == 0 and x[0] > MS]
                                + [mk(x) for x in pending if x[2] // 2 == 1],
                            ]
                            add(h + 1, 1, piece0)
                            add(h + 1, 2, piece1)
                            return carry_out
                        # last head: drain piece-major, readout each quarter
                        # as soon as its AV accumulation is done
                        pending.sort(key=lambda x: x[2])
                        done_upto = 0
                        for x in pending:
                            while done_upto < x[2]:
                                piece(done_upto, quarter=True)
                                done_upto += 1
                            emit_av_piece(x[1], x[2], x[3])
                        while done_upto < 4:
                            piece(done_upto, quarter=True)
                            done_upto += 1
                        return []

                    carry = []
                    for h in range(NHC):
                        carry = attention_head(h, carry)
                    assert not sched, f"unemitted quanta: {list(sched)}"

    nc.finalize()
    return nc


_NC_CACHE = {}


def _get_nc(iters=1):
    if iters not in _NC_CACHE:
        _NC_CACHE[iters] = _build(iters)
    return _NC_CACHE[iters]


def _permute_wqk(wqk):
    # [k*128+p, m*128+j] -> [m*128+p, k*128+j]: one m-tile contiguous per row
    w4 = wqk.reshape(KD, P, KD, P)
    return np.ascontiguousarray(w4.transpose(2, 1, 0, 3).reshape(D, D))


def make_in_maps(inputs, W_qkv, b_qkv):
    inputs = np.asarray(inputs, dtype=np.float32)
    W = np.asarray(W_qkv, dtype=np.float32)
    b = np.asarray(b_qkv, dtype=np.float32)
    xt_by_b = [np.ascontiguousarray(inputs[bi].T).astype(np.float16)
               for bi in range(B_FULL)]
    in_maps = []
    for c in range(N_CORES):
        bi, hg = c // 2, c % 2
        c0 = hg * QKC
        in_maps.append({
            "xt": xt_by_b[bi],
            "wqk": _permute_wqk(
                np.concatenate([W[:, c0:c0 + QKC], W[:, D + c0: D + c0 + QKC]],
                               axis=1)).astype(np.float16),
            "wv": np.ascontiguousarray(
                W[:, 2 * D + c0: 2 * D + c0 + QKC]).astype(np.float16),
            "bqk": np.ascontiguousarray(
                np.concatenate([b[c0:c0 + QKC], b[D + c0: D + c0 + QKC]])),
            "bv": np.ascontiguousarray(b[2 * D + c0: 2 * D + c0 + QKC]),
        })
    return in_maps


def assemble(results, B=B_FULL):
    out = np.empty((B, S, D), dtype=np.float32)
    for c in range(N_CORES):
        bi, hg = c // 2, c % 2
        numer = np.asarray(results[c]["out"])           # [S, 512]
        sums = np.asarray(results[c]["sums"])           # [S, 8]
        out[bi, :, hg * QKC:(hg + 1) * QKC] = (
            numer.reshape(S, NHC, HD) / sums[:, :, None]).reshape(S, QKC)
    return out


def kernel(inputs, mask, W_qkv, b_qkv):
    # mask is all-True for this problem (spec: fill=ones); it does not affect softmax.
    nc = _get_nc()
    in_maps = make_in_maps(inputs, W_qkv, b_qkv)
    res = run_bass_kernel_spmd(nc, in_maps, core_ids=list(range(N_CORES)))
    return assemble(res.results)
